# revision 1
# baseline (speedup 1.0000x reference)
"""Trainium2 Bass kernel for nn_ContentOnlyModel (embedding_lookup).

Model: score[b,t] = MLP(LN(txt_table[id]), LN(img_table[id])) — a pure
per-id function.  Host folds LN into the tables (row-wise, id-independent),
concatenates txt+img rows into one [V, 1280] fp16 table, and dedupes the
51200 requested ids.  The 8 cores are vocab-parallel: core k holds rows
[k*12501, (k+1)*12501) so dma_gather's int16 indices are in range.  Each
core gathers its unique ids with a transposing dma_gather (row value d
lands at partition d%128, chunk d//128 — exactly the matmul contraction
layout), then runs the 3-layer MLP on PE/ACT.  Host scatters the per-id
scores back to token positions, adds the final bias, and masks id==0.
"""

import sys

for _p in ("/opt/trn_rl_repo",):
    if _p not in sys.path:
        sys.path.insert(0, _p)

import numpy as np

import concourse.bacc as bacc
import concourse.mybir as mybir
import concourse.tile as tile
from concourse.bass_utils import run_bass_kernel_spmd

N_CORES = 8
I_FULL = 100001          # vocab rows
DT, DI = 768, 512        # txt/img dims
D_COMB = DT + DI         # 1280
NCH = D_COMB // 128      # 10 contraction chunks
HM, H = 64, 128
V8 = 12501               # rows per core shard (8*12501 = 100008 >= 100001)
CH = 512                 # ids per pipeline chunk
EPS = 1e-5

_nc_cache: dict[int, object] = {}


def build_nc(n_pad: int, ch: int = CH, xt_bufs: int = 4, h_bufs: int = 3,
             ps_bufs: int = 2, wstat: int = 3, nq: int = 1,
             scratch: int = 16384, strip: int = 256, lookahead: int = 2):
    """Device program: gather n_pad ids from the local table shard and
    score them.  Shared by all 8 cores (SPMD).

    wstat>1 groups that many token-chunks per weight pass (weight-stationary
    over the group, fewer LDWEIGHTS)."""
    assert n_pad % ch == 0
    n_chunks = n_pad // ch
    f16, f32, i16 = mybir.dt.float16, mybir.dt.float32, mybir.dt.int16

    nc = bacc.Bacc("TRN2", target_bir_lowering=False, debug=False,
                   num_devices=N_CORES, num_swdge_queues=nq,
                   dynamic_dma_scratch_size=scratch)
    table = nc.dram_tensor("table", [V8, D_COMB], f16, kind="ExternalInput")
    idxs = nc.dram_tensor("idxs", [128, n_pad // 16], i16, kind="ExternalInput")
    w1 = nc.dram_tensor("w1", [128, NCH, 128], f16, kind="ExternalInput")
    w2 = nc.dram_tensor("w2", [128, 128], f16, kind="ExternalInput")
    w3 = nc.dram_tensor("w3", [128, 8], f16, kind="ExternalInput")
    bias = nc.dram_tensor("bias", [128, 2], f32, kind="ExternalInput")
    out = nc.dram_tensor("out", [1, n_pad], f32, kind="ExternalOutput")

    relu = mybir.ActivationFunctionType.Relu

    with tile.TileContext(nc) as tc:
        with (
            tc.tile_pool(name="const", bufs=1) as cpool,
            tc.tile_pool(name="x", bufs=xt_bufs) as xpool,
            tc.tile_pool(name="h", bufs=h_bufs) as hpool,
            tc.tile_pool(name="ps", bufs=ps_bufs, space="PSUM") as pspool,
            tc.tile_pool(name="ps1g", bufs=wstat + 1, space="PSUM") as ps1pool,
            tc.tile_pool(name="ob", bufs=1) as opool,
        ):
            w1_t = cpool.tile([128, NCH, 128], f16)
            w2_t = cpool.tile([128, 128], f16)
            w3_t = cpool.tile([128, 8], f16)
            bias_t = cpool.tile([128, 2], f32)
            idx_t = cpool.tile([128, n_pad // 16], i16)
            first_cols = min(CH // 16, n_pad // 16)
            nc.sync.dma_start(out=idx_t[:, :first_cols],
                              in_=idxs[:, :first_cols])
            if n_pad // 16 > first_cols:
                nc.sync.dma_start(out=idx_t[:, first_cols:],
                                  in_=idxs[:, first_cols:])
            nc.sync.dma_start(out=w1_t[:], in_=w1[:])
            nc.sync.dma_start(out=w2_t[:], in_=w2[:])
            nc.sync.dma_start(out=w3_t[:], in_=w3[:])
            nc.sync.dma_start(out=bias_t[:], in_=bias[:])

            # PE warmup: dummy matmuls release the HAM clock gate during the
            # initial gather latency so real matmuls start at full clock.
            wu_rhs = cpool.tile([128, 512], f16)
            nc.vector.memset(wu_rhs[:], 0)
            wu_ps = pspool.tile([128, 512], f32, tag="ps2", name="wups")
            for _ in range(16):
                nc.tensor.matmul(wu_ps[:], lhsT=wu_rhs[:, :128],
                                 rhs=wu_rhs[:], start=True, stop=True)

            ob_all = opool.tile([1, n_pad], f32)

            # gather units: uniform ch-row gathers, except the final ch is
            # split into two strip-sized gathers so the drain chain starts
            # before the last bytes land.  compute units: one per gather,
            # with the tail gathers stripped for a short drain chain.
            if n_pad // ch >= 3 and ch == 2 * strip:
                g_sizes = [ch] * (n_pad // ch - 2) + [strip] * 4
            elif n_pad // ch >= 2 and ch == 2 * strip:
                g_sizes = [ch] * (n_pad // ch - 1) + [strip, strip]
            else:
                g_sizes = [ch] * (n_pad // ch)
            g_offs = [sum(g_sizes[:i]) for i in range(len(g_sizes))]
            n_g = len(g_sizes)
            c_units = []  # (gather_idx, col_offset, size)
            for gi in range(n_g):
                if g_sizes[gi] == ch and gi < n_g - 1:
                    c_units.append((gi, 0, ch))
                else:
                    for s in range(0, g_sizes[gi], strip):
                        c_units.append((gi, s, strip))
            n_cu = len(c_units)
            users_left = {gi: sum(1 for g, _, _ in c_units if g == gi)
                          for gi in range(n_g)}

            xts, ps1s, h1s, ps2s, h2s, ps3s = {}, {}, {}, {}, {}, {}

            def gather(gi):
                gsz = g_sizes[gi]
                xt = xpool.tile([128, NCH, gsz], f16, tag="xt", name="xt")
                nc.gpsimd.dma_gather(
                    xt[:], table[:],
                    idx_t[:, g_offs[gi] // 16:(g_offs[gi] + gsz) // 16],
                    gsz, gsz, D_COMB, transpose=True, queue_num=gi % nq)
                xts[gi] = xt

            m1_last, m2_inst = {}, {}

            def m1(cu):
                gi, co, sz = c_units[cu]
                ps1s[cu] = ps1pool.tile([128, sz], f32, tag="ps1", name="ps1")
                for c in range(NCH):
                    m1_last[cu] = nc.tensor.matmul(
                        ps1s[cu][:], lhsT=w1_t[:, c, :],
                        rhs=xts[gi][:, c, co:co + sz],
                        start=(c == 0), stop=(c == NCH - 1))
                users_left[gi] -= 1
                if users_left[gi] == 0:
                    del xts[gi]

            def a1(cu):
                sz = c_units[cu][2]
                h1s[cu] = hpool.tile([128, sz], f16, tag="h1", name="h1")
                nc.scalar.activation(h1s[cu][:], ps1s[cu][:], relu,
                                     bias=bias_t[:, 0:1])
                del ps1s[cu]

            def m2(cu):
                sz = c_units[cu][2]
                ps2s[cu] = pspool.tile([128, sz], f32, tag="ps2", name="ps2")
                m2_inst[cu] = nc.tensor.matmul(ps2s[cu][:], lhsT=w2_t[:],
                                 rhs=h1s[cu][:], start=True, stop=True)
                if cu + 1 in m1_last:
                    tile.add_dep_helper(m2_inst[cu].ins, m1_last[cu + 1].ins,
                                        sync=False,
                                        reason="pipeline: M2_j after M1_j+1")
                del h1s[cu]

            def a2(cu):
                sz = c_units[cu][2]
                h2s[cu] = hpool.tile([128, sz], f16, tag="h2", name="h2")
                nc.scalar.activation(h2s[cu][:], ps2s[cu][:], relu,
                                     bias=bias_t[:, 1:2])
                del ps2s[cu]

            def m3(cu):
                sz = c_units[cu][2]
                ps3s[cu] = pspool.tile([1, sz], f32, tag="ps3", name="ps3", bufs=1)
                inst = nc.tensor.matmul(ps3s[cu][:], lhsT=w3_t[:, 0:1],
                                 rhs=h2s[cu][:], start=True, stop=True)
                if cu + 1 in m2_inst:
                    tile.add_dep_helper(inst.ins, m2_inst[cu + 1].ins, sync=False,
                                        reason="pipeline: M3_j after M2_j+1")
                del h2s[cu]

            def cp(cu):
                gi, co, sz = c_units[cu]
                off = g_offs[gi] + co
                nc.vector.tensor_copy(ob_all[:, off:off + sz], ps3s[cu][:])
                del ps3s[cu]

            issued = 0

            def issue_gathers(upto):
                nonlocal issued
                while issued < min(upto, n_g):
                    gather(issued)
                    issued += 1

            issue_gathers(lookahead)
            for j in range(n_cu + 2):
                if j < n_cu:
                    issue_gathers(c_units[j][0] + 1 + lookahead)
                    m1(j)
                if 1 <= j <= n_cu:
                    m2(j - 1)
                if 2 <= j:
                    m3(j - 2)
                if j < n_cu:
                    a1(j)
                if 1 <= j <= n_cu:
                    a2(j - 1)
                if 2 <= j:
                    cp(j - 2)
                if j == n_cu:
                    last_off = n_pad - strip
                    nc.sync.dma_start(out=out[0:1, :last_off],
                                      in_=ob_all[:, :last_off])

            last_off = n_pad - strip
            nc.sync.dma_start(out=out[0:1, last_off:],
                              in_=ob_all[:, last_off:])

    nc.compile()
    return nc


def _prep_host(inputs):
    """Fold LN + layer1 layout on host; returns (comb_table_f16, weight
    arrays)."""
    txt = np.asarray(inputs["txt_table"], np.float32)
    img = np.asarray(inputs["img_table"], np.float32)

    def ln(x, g, b):
        mu = x.mean(axis=1, keepdims=True)
        xc = x - mu
        var = (xc * xc).mean(axis=1, keepdims=True)
        return xc * (1.0 / np.sqrt(var + EPS)) * g + b

    txt_n = ln(txt, np.asarray(inputs["ln_txt_g"], np.float32),
               np.asarray(inputs["ln_txt_b"], np.float32))
    img_n = ln(img, np.asarray(inputs["ln_img_g"], np.float32),
               np.asarray(inputs["ln_img_b"], np.float32))

    comb = np.zeros((N_CORES * V8, D_COMB), np.float16)
    comb[:I_FULL, :DT] = txt_n
    comb[:I_FULL, DT:] = img_n

    # lhsT layer1: [d_in_chunk(128 part), chunk, h] ; block diagonal
    txt_w = np.asarray(inputs["txt_w"], np.float32)   # [64, 768]
    img_w = np.asarray(inputs["img_w"], np.float32)   # [64, 512]
    w_comb = np.zeros((D_COMB, H), np.float32)
    w_comb[:DT, :HM] = txt_w.T
    w_comb[DT:, HM:] = img_w.T
    w1_dram = np.ascontiguousarray(
        w_comb.reshape(NCH, 128, H).transpose(1, 0, 2)).astype(np.float16)

    w2_dram = np.asarray(inputs["fus_w1"], np.float32).T.astype(np.float16)
    w3_dram = np.zeros((128, 8), np.float16)
    w3_dram[:, 0] = np.asarray(inputs["fus_w2"], np.float32)[0]
    bias_dram = np.zeros((128, 2), np.float32)
    bias_dram[:, 0] = np.concatenate([
        np.asarray(inputs["txt_bias"], np.float32),
        np.asarray(inputs["img_bias"], np.float32),
    ])
    bias_dram[:, 1] = np.asarray(inputs["fus_b1"], np.float32)
    return comb, w1_dram, w2_dram, w3_dram, bias_dram


def _wrap_idxs(local: np.ndarray, n_pad: int) -> np.ndarray:
    """idx i -> partition i%16, column i//16; replicated to 128 partitions."""
    padded = np.zeros(n_pad, np.int16)
    padded[:len(local)] = local
    tile16 = padded.reshape(n_pad // 16, 16).T  # [16, n_pad//16]
    return np.ascontiguousarray(np.tile(tile16, (8, 1)))


def kernel(**inputs):
    pos = np.asarray(inputs["pos_seqs"])
    neg = np.asarray(inputs["neg_seqs"])
    B, T = pos.shape

    comb, w1_dram, w2_dram, w3_dram, bias_dram = _prep_host(inputs)

    ids_all = np.concatenate([pos.ravel(), neg.ravel()]).astype(np.int64)
    uniq, inv = np.unique(ids_all, return_inverse=True)
    bounds = np.searchsorted(uniq, np.arange(1, N_CORES) * V8)
    segs = np.split(uniq, bounds)
    counts = [len(s) for s in segs]
    n_pad = max(CH, -(-max(counts) // CH) * CH)

    in_maps = []
    for k in range(N_CORES):
        local = (segs[k] - k * V8).astype(np.int16)
        in_maps.append({
            "table": np.ascontiguousarray(comb[k * V8:(k + 1) * V8]),
            "idxs": _wrap_idxs(local, n_pad),
            "w1": w1_dram,
            "w2": w2_dram,
            "w3": w3_dram,
            "bias": bias_dram,
        })

    nc = _nc_cache.get(n_pad)
    if nc is None:
        nc = build_nc(n_pad)
        _nc_cache[n_pad] = nc

    res = None
    for attempt in range(3):
        try:
            res = run_bass_kernel_spmd(nc, in_maps,
                                       core_ids=list(range(N_CORES)))
            break
        except Exception:
            # transient NRT_EXEC_UNIT_UNRECOVERABLE has been observed on the
            # axon workers; a clean retry succeeds
            if attempt == 2:
                raise
            import time
            time.sleep(5)
            try:
                import jax
                jax.clear_backends()
            except Exception:
                pass

    score_uniq = np.concatenate(
        [res.results[k]["out"][0, :counts[k]] for k in range(N_CORES)])
    fus_b2 = float(np.asarray(inputs["fus_b2"], np.float32)[0])
    scores = score_uniq[inv].astype(np.float32) + fus_b2
    scores[ids_all == 0] = 0.0
    n_tok = B * T
    pos_out = scores[:n_tok].reshape(B, T)
    neg_out = scores[n_tok:].reshape(B, T)
    return pos_out, neg_out

